# revision 20
# baseline (speedup 1.0000x reference)
"""Trainium2 Bass kernel for nn_Encoder_41412074668419 (ragged_sequence).

Strategy: data-parallel over batch B=8 across 8 NeuronCores (one sample per
core).  Host side does only index/layout prep: extracting the ragged-gather
indices from the mask, gathering the per-observation scalars, and building the
one-hot segment matrices (M_t [L,N] / M_c [E,N]) that the device uses to do
gathers and segment-softmax sums as matmuls.

Math reformulation (validated bit-close against the jax reference):
  * The mask has exactly N=2624 ones per row, so the gathered mask is all-ones
    and the attention masks are pure one-hot segment selectors.
  * Masked softmax == segment softmax:
      s[n,h] = (lin(gathered_emb, q) . k[n])_h / sqrt(NK)     (per-observation)
      P = exp(s)                      (scores are tiny; no max subtraction)
      denom[seg,h] = sum_{n in seg} P[n,h]      == M @ P      (matmul)
      numer[seg,:] = sum_{n in seg} P[n,h]*v[n] == M @ (P*v)  (matmul)
      o = q + numer/denom + bias_v              (v-bias rides through softmax)
  * Row gathers emb[idx] are one-hot matmuls; linear biases ride through them.

All matmul operands are fp16 (PE 1 cyc/row, fp32 PSUM accumulate).
"""

import numpy as np

import concourse.bacc as bacc
import concourse.bass as bass
import concourse.mybir as mybir
import concourse.tile as tile
from concourse.bass_utils import run_bass_kernel_spmd

B, L, E = 8, 256, 41
NK, NH, NL = 128, 4, 3
HD = NK // NH
N_OBS = 2624
NP = 2688                 # padded observation count: 21 * 128
NCH = NP // 128           # 21 partition chunks of the n dimension
INV_SQRT_NK = 1.0 / float(np.sqrt(NK))
F16 = mybir.dt.float16
F32 = mybir.dt.float32
ID = mybir.ActivationFunctionType.Identity
COLS = [(o, min(512, NP - o)) for o in range(0, NP, 512)]   # 512-wide col chunks

_TRACE = False            # test.py flips this to profile
_PROGRAM = None           # cached (nc, input name list)


def _build_program():
    nc = bacc.Bacc("TRN2", target_bir_lowering=False, debug=False)

    def din(name, shape, dt=F16):
        return nc.declare_dram_parameter(name, list(shape), dt, isOutput=False)

    # ---- per-core ragged data ----
    d_vfeat = din("vfeat", [2, NP])                    # [value_f; tm_f]
    d_cx = din("cx", [1, L])
    d_mt = din("mt", [128, 2, NP])                     # M_t   [t_in_chunk, t_chunk, n]
    d_mtt = din("mtt", [128, NCH, L])                  # M_t^T [n_in_chunk, n_chunk, t]
    d_mc = din("mc", [E, NP])                          # M_c   [e, n]
    d_mct = din("mct", [128, NCH, E])                  # M_c^T [n_in_chunk, n_chunk, e]
    # ---- replicated consts ----
    d_ce0b = din("ce0b", [128, E])                     # initial chan emb, [f, e]
    d_ce0a = din("ce0a", [E, 128])                     # initial chan emb, [e, f]
    d_ident = din("ident", [128, 128])
    d_onesh = din("onesh", [128, NH])                  # ones_heads[f, h] = (f//HD == h)
    d_oneshT = din("oneshT", [NH, 128])
    d_einit_w = din("einit_w", [2, 128])
    d_einit_b = din("einit_b", [128, 1], F32)
    d_time_w = din("time_w", [1, 128])
    d_time_b = din("time_b", [128, 1], F32)
    d_outp_w = din("outp_w", [128, 3])                 # W_out [384,1] chunked [c*128+p] -> [p, c]
    d_outp_b = din("outp_b", [1, 1], F32)
    mabs = {}
    for l in range(NL):
        for m in ("ct", "tc"):
            p = f"{m}{l}"
            mabs[p] = dict(
                wq=din(f"wq_{p}", [128, 128]),
                wk=din(f"wk_{p}", [128, 2, 128]),      # [f_in_half, half, f']
                wv=din(f"wv_{p}", [128, 2, 128]),
                wo=din(f"wo_{p}", [128, 128]),
                bq=din(f"bq_{p}", [128, 1], F32),
                bk=din(f"bk_{p}", [128, 1], F32),
                bv=din(f"bv_{p}", [128, 1], F32),
                bo=din(f"bo_{p}", [128, 1], F32),
            )
    d_we = [din(f"we_{l}", [128, 3, 128]) for l in range(NL)]
    d_be = [din(f"be_{l}", [128, 1], F32) for l in range(NL)]

    d_out = nc.declare_dram_parameter("out", [1, NP], F32, isOutput=True)

    # PSUM is 8 banks; every psum tile occupies >=1 full bank:
    #   "mm" 3 + "sm" 3 + numer 1 + denom 1 = 8
    with tile.TileContext(nc) as tc, \
         tc.tile_pool(name="consts", bufs=1) as consts, \
         tc.tile_pool(name="acts", bufs=2) as acts, \
         tc.tile_pool(name="small", bufs=3) as small, \
         tc.tile_pool(name="psum", bufs=3, space="PSUM") as psum, \
         tc.tile_pool(name="psum_sm", bufs=3, space="PSUM") as psum_sm, \
         tc.tile_pool(name="psum_acc", bufs=1, space="PSUM") as psum_acc:

        def cload(dram, shape, dt=F16, tag=None):
            t = consts.tile(shape, dt, tag=tag or dram.name)
            nc.sync.dma_start(out=t[:], in_=dram[:])
            return t

        # ---------- load constants ----------
        vfeat = cload(d_vfeat, [2, NP])
        cx = cload(d_cx, [1, L])
        mt = cload(d_mt, [128, 2, NP])
        mtt = cload(d_mtt, [128, NCH, L])
        mc = cload(d_mc, [E, NP])
        mct = cload(d_mct, [128, NCH, E])
        ce0b = cload(d_ce0b, [128, E])
        ce0a = cload(d_ce0a, [E, 128])
        ident = cload(d_ident, [128, 128])
        onesh = cload(d_onesh, [128, NH])
        oneshT = cload(d_oneshT, [NH, 128])
        einit_w = cload(d_einit_w, [2, 128])
        einit_b = cload(d_einit_b, [128, 1], F32)
        time_w = cload(d_time_w, [1, 128])
        time_b = cload(d_time_b, [128, 1], F32)
        outp_w = cload(d_outp_w, [128, 3])
        outp_b = cload(d_outp_b, [1, 1], F32)
        W = {}
        for p, d in mabs.items():
            W[p] = {k: cload(v, list(v.shape), v.dtype) for k, v in d.items()}
        we = [cload(d_we[l], [128, 3, 128]) for l in range(NL)]
        be = [cload(d_be[l], [128, 1], F32) for l in range(NL)]

        AF = mybir.ActivationFunctionType
        OP = mybir.AluOpType

        # ---------- init ----------
        # time emb: sin(cx @ W + b)   [f, t] and transposed [t, f]
        te_b = acts.tile([128, L], F16, tag="te_b")
        ps = psum_sm.tile([128, L], F32, tag="sm")
        nc.tensor.matmul(ps[:], lhsT=time_w[:], rhs=cx[:], start=True, stop=True)
        nc.scalar.activation(out=te_b[:], in_=ps[:], func=AF.Sin, bias=time_b[:])

        def transpose_b_to_a(src_b, S, tag):
            """[128, S] -> [S(part chunks), 128] as tile [128, ceil(S/128), 128]."""
            nchunk = (S + 127) // 128
            dst = acts.tile([128, nchunk, 128], F16, tag=tag)
            for c in range(nchunk):
                w = min(128, S - c * 128)
                tp = psum_sm.tile([128, 128], F16, tag="sm")
                nc.tensor.transpose(
                    tp[:w, :], in_=src_b[:, c * 128:c * 128 + w], identity=ident[:]
                )
                nc.vector.tensor_copy(out=dst[:w, c, :], in_=tp[:w, :])
            return dst

        te_a = transpose_b_to_a(te_b, L, "te_a")
        ce_b = ce0b
        ce_a_ap = ce0a[:]

        # value feature init: relu(vfeat @ W + b), [f, n]
        vf = acts.tile([128, NP], F16, tag="vf")
        for o, w in COLS:
            ps = psum.tile([128, 512], F32, tag="mm")
            nc.tensor.matmul(ps[:, :w], lhsT=einit_w[:], rhs=vfeat[:, o:o + w],
                             start=True, stop=True)
            nc.scalar.activation(out=vf[:, o:o + w], in_=ps[:, :w], func=AF.Relu,
                                 bias=einit_b[:])

        def gather_time(src_a, tag):
            """k_t[f, n] = sum_t src_a[t, f] * M_t[t, n]"""
            out = acts.tile([128, NP], F16, tag=tag)
            for o, w in COLS:
                ps = psum.tile([128, 512], F32, tag="mm")
                for c in range(2):
                    nc.tensor.matmul(ps[:, :w], lhsT=src_a[:, c, :],
                                     rhs=mt[:, c, o:o + w],
                                     start=(c == 0), stop=(c == 1))
                nc.scalar.copy(out=out[:, o:o + w], in_=ps[:, :w])
            return out

        def gather_chan(src_a_ap, tag):
            """k_c[f, n] = sum_e src_a[e, f] * M_c[e, n];  src_a_ap: [E, 128]."""
            out = acts.tile([128, NP], F16, tag=tag)
            for o, w in COLS:
                ps = psum.tile([128, 512], F32, tag="mm")
                nc.tensor.matmul(ps[:, :w], lhsT=src_a_ap,
                                 rhs=mc[:, o:o + w], start=True, stop=True)
                nc.vector.tensor_copy(out=out[:, o:o + w], in_=ps[:, :w])
            return out

        def mab(wp, q_emb_b, S, qg_src, k_first, m_T):
            """One masked-attention block (segment form).

            wp: weight dict.  q_emb_b: [128, S] query-side embedding (layout b).
            qg_src: [128, NP] query emb gathered per observation.
            k_first: [128, NP] first half of the K-side concat (second is vf).
            m_T: [128, NCH, S] one-hot M^T for numer/denom contraction.
            Returns new embedding in both layouts.
            """
            # q = lin(q_emb) (+bq)
            q_sb = small.tile([128, S], F16, tag="q_sb")
            ps = psum_sm.tile([128, S], F32, tag="sm")
            nc.tensor.matmul(ps[:], lhsT=wp["wq"][:], rhs=q_emb_b[:, :S],
                             start=True, stop=True)
            nc.scalar.activation(out=q_sb[:], in_=ps[:], func=ID, bias=wp["bq"][:])

            # qg = lin(gathered q emb) (+bq)  [f, n]
            qg = acts.tile([128, NP], F16, tag="qg")
            for o, w in COLS:
                ps = psum.tile([128, 512], F32, tag="mm")
                nc.tensor.matmul(ps[:, :w], lhsT=wp["wq"][:], rhs=qg_src[:, o:o + w],
                                 start=True, stop=True)
                nc.scalar.activation(out=qg[:, o:o + w], in_=ps[:, :w], func=ID,
                                     bias=wp["bq"][:])

            # k = lin([k_first; vf]) + bk ; prod = k * qg  [f, n]
            k_sb = acts.tile([128, NP], F16, tag="k_sb")
            prod = acts.tile([128, NP], F16, tag="prod")
            for o, w in COLS:
                ps = psum.tile([128, 512], F32, tag="mm")
                nc.tensor.matmul(ps[:, :w], lhsT=wp["wk"][:, 0, :],
                                 rhs=k_first[:, o:o + w], start=True, stop=False)
                nc.tensor.matmul(ps[:, :w], lhsT=wp["wk"][:, 1, :],
                                 rhs=vf[:, o:o + w], start=False, stop=True)
                nc.scalar.activation(out=k_sb[:, o:o + w], in_=ps[:, :w], func=ID,
                                     bias=wp["bk"][:])
                nc.vector.tensor_mul(prod[:, o:o + w], k_sb[:, o:o + w],
                                     qg[:, o:o + w])

            # v in layout a: [n(part chunks), f']  (no bias: bv applied post-softmax)
            va = acts.tile([128, NCH, 128], F16, tag="va")
            for c in range(NCH):
                ps = psum.tile([128, 512], F32, tag="mm")
                nc.tensor.matmul(ps[:, :128], lhsT=k_first[:, c * 128:(c + 1) * 128],
                                 rhs=wp["wv"][:, 0, :], start=True, stop=False)
                nc.tensor.matmul(ps[:, :128], lhsT=vf[:, c * 128:(c + 1) * 128],
                                 rhs=wp["wv"][:, 1, :], start=False, stop=True)
                nc.scalar.copy(out=va[:, c, :], in_=ps[:, :128])

            # s4 = per-head dot; P4 = exp(s/sqrt(NK))   [h, n]
            p4 = acts.tile([NH, NP], F16, tag="p4")
            for o, w in COLS:
                ps = psum_sm.tile([NH, 512], F32, tag="sm")
                nc.tensor.matmul(ps[:, :w], lhsT=onesh[:], rhs=prod[:, o:o + w],
                                 start=True, stop=True)
                nc.scalar.activation(out=p4[:, o:o + w], in_=ps[:, :w], func=AF.Exp,
                                     scale=INV_SQRT_NK)

            # P in layout a: [n(part chunks), h]
            pa = small.tile([128, NCH, NH], F16, tag="pa")
            for c in range(NCH):
                tp = psum_sm.tile([128, NH], F16, tag="sm")
                nc.tensor.transpose(tp[:], in_=p4[:, c * 128:(c + 1) * 128],
                                    identity=ident[:NH, :NH])
                nc.vector.tensor_copy(out=pa[:, c, :], in_=tp[:])

            # PV = v * P (broadcast P over head dim)
            pv = acts.tile([128, NCH, 128], F16, tag="pv")
            for c in range(NCH):
                nc.vector.tensor_tensor(
                    out=pv[:, c, :].rearrange("p (h d) -> p h d", h=NH),
                    in0=va[:, c, :].rearrange("p (h d) -> p h d", h=NH),
                    in1=pa[:, c, :, None].to_broadcast((128, NH, HD)),
                    op=OP.mult)

            # numer [f', seg] and denom [h, seg] via one-hot contraction over n
            ps_n = psum_acc.tile([128, 256], F32, tag="numer")
            ps_d = psum_acc.tile([NH, 256], F32, tag="denom")
            for c in range(NCH):
                nc.tensor.matmul(ps_n[:, :S], lhsT=pv[:, c, :], rhs=m_T[:, c, :S],
                                 start=(c == 0), stop=(c == NCH - 1))
            for c in range(NCH):
                nc.tensor.matmul(ps_d[:, :S], lhsT=pa[:, c, :], rhs=m_T[:, c, :S],
                                 start=(c == 0), stop=(c == NCH - 1))

            # o = q + numer/denom + bv ; out = o + relu(lin(o) + bo)
            rd = small.tile([NH, S], F16, tag="rd")
            with nc.allow_low_precision("softmax denoms are O(segment size); fp16 ok"):
                nc.vector.reciprocal(out=rd[:], in_=ps_d[:, :S])
            ps_rb = psum_sm.tile([128, S], F32, tag="sm")
            nc.tensor.matmul(ps_rb[:], lhsT=oneshT[:], rhs=rd[:], start=True, stop=True)
            rb = small.tile([128, S], F16, tag="rb")
            nc.vector.tensor_copy(out=rb[:], in_=ps_rb[:])
            t0 = small.tile([128, S], F32, tag="t0")
            nc.vector.tensor_mul(t0[:], ps_n[:, :S], rb[:])
            t0b = small.tile([128, S], F32, tag="t0b")
            nc.scalar.activation(out=t0b[:], in_=t0[:], func=ID, bias=wp["bv"][:])
            o_pre = small.tile([128, S], F16, tag="o_pre")
            nc.vector.tensor_add(o_pre[:], t0b[:], q_sb[:])
            ps_o = psum_sm.tile([128, S], F32, tag="sm")
            nc.tensor.matmul(ps_o[:], lhsT=wp["wo"][:], rhs=o_pre[:], start=True, stop=True)
            t1 = small.tile([128, S], F16, tag="t1")
            nc.scalar.activation(out=t1[:], in_=ps_o[:], func=AF.Relu, bias=wp["bo"][:])
            new_b = acts.tile([128, S], F16, tag="new_b")
            nc.vector.tensor_add(new_b[:], o_pre[:], t1[:])
            new_a = transpose_b_to_a(new_b, S, "new_a")
            return new_b, new_a

        # ---------- layers ----------
        for l in range(NL):
            k_t = gather_time(te_a, "k_t")
            k_c = gather_chan(ce_a_ap, "k_c")

            c_b, c_a = mab(W[f"ct{l}"], ce_b, E, k_c, k_t, mct)
            t_b, t_a = mab(W[f"tc{l}"], te_b, L, k_t, k_c, mtt)

            # vf = relu(vf + lin([vf; k_t; k_c]) + be)
            vf_new = acts.tile([128, NP], F16, tag="vf")
            srcs = (vf, k_t, k_c)
            for o, w in COLS:
                ps = psum.tile([128, 512], F32, tag="mm")
                for c in range(3):
                    nc.tensor.matmul(ps[:, :w], lhsT=we[l][:, c, :],
                                     rhs=srcs[c][:, o:o + w],
                                     start=(c == 0), stop=(c == 2))
                t2 = small.tile([128, 512], F32, tag="t_edge")
                nc.vector.tensor_add(t2[:, :w], ps[:, :w], vf[:, o:o + w])
                nc.scalar.activation(out=vf_new[:, o:o + w], in_=t2[:, :w],
                                     func=AF.Relu, bias=be[l][:])
            vf = vf_new
            ce_b = c_b
            ce_a_ap = c_a[:E, 0, :]
            te_b, te_a = t_b, t_a

        # ---------- output ----------
        k_t = gather_time(te_a, "k_t")
        k_c = gather_chan(ce_a_ap, "k_c")
        out_sb = acts.tile([1, NP], F32, tag="out_sb")
        srcs = (vf, k_t, k_c)
        for o, w in COLS:
            ps = psum_sm.tile([1, 512], F32, tag="sm")
            for c in range(3):
                nc.tensor.matmul(ps[:, :w], lhsT=outp_w[:, c:c + 1],
                                 rhs=srcs[c][:, o:o + w],
                                 start=(c == 0), stop=(c == 2))
            nc.scalar.activation(out=out_sb[:, o:o + w], in_=ps[:, :w], func=ID,
                                 bias=outp_b[:])
        nc.sync.dma_start(out=d_out[:], in_=out_sb[:])

    nc.compile()
    return nc


def _get_program():
    global _PROGRAM
    if _PROGRAM is None:
        _PROGRAM = _build_program()
    return _PROGRAM


def _prep_core_inputs(b, context_x, value, target_value, target_mask, idx):
    tif = idx // E
    cif = idx % E
    n = idx.shape[0]
    vfeat = np.zeros((2, NP), np.float16)
    vfeat[0, :n] = value[b].reshape(-1)[idx]
    vfeat[1, :n] = target_mask[b].reshape(-1)[idx]
    mtf = np.zeros((L, NP), np.float16)
    mtf[tif, np.arange(n)] = 1.0
    mcf = np.zeros((E, NP), np.float16)
    mcf[cif, np.arange(n)] = 1.0
    return {
        "vfeat": vfeat,
        "cx": context_x[b].reshape(1, L).astype(np.float16),
        "mt": np.ascontiguousarray(mtf.reshape(2, 128, NP).transpose(1, 0, 2)),
        "mtt": np.ascontiguousarray(mtf.T.reshape(NCH, 128, L).transpose(1, 0, 2)),
        "mc": mcf,
        "mct": np.ascontiguousarray(mcf.T.reshape(NCH, 128, E).transpose(1, 0, 2)),
    }


def _shared_inputs(params):
    a = lambda x: np.asarray(x)
    f16 = lambda x: np.asarray(x, np.float16)
    col = lambda x: np.asarray(x, np.float32).reshape(-1, 1)
    sh = {}
    wc, bc = params["chan_init"]
    ce0 = np.maximum(a(wc) + a(bc)[None, :], 0.0)           # [E, NK]
    sh["ce0b"] = f16(ce0.T)
    sh["ce0a"] = f16(ce0)
    sh["ident"] = np.eye(128, dtype=np.float16)
    oh = np.zeros((128, NH), np.float16)
    for h in range(NH):
        oh[h * HD:(h + 1) * HD, h] = 1.0
    sh["onesh"] = oh
    sh["oneshT"] = np.ascontiguousarray(oh.T)
    sh["einit_w"] = f16(params["edge_init"][0])
    sh["einit_b"] = col(params["edge_init"][1])
    sh["time_w"] = f16(params["time_init"][0])
    sh["time_b"] = col(params["time_init"][1])
    sh["outp_w"] = np.ascontiguousarray(f16(params["output"][0]).reshape(3, 128).T)
    sh["outp_b"] = col(params["output"][1])
    for l, lp in enumerate(params["layers"]):
        for m in ("ct", "tc"):
            mp, p = lp[m], f"{m}{l}"
            sh[f"wq_{p}"] = f16(mp["q"][0])
            sh[f"wk_{p}"] = np.ascontiguousarray(f16(mp["k"][0]).reshape(2, 128, 128).transpose(1, 0, 2))
            sh[f"wv_{p}"] = np.ascontiguousarray(f16(mp["v"][0]).reshape(2, 128, 128).transpose(1, 0, 2))
            sh[f"wo_{p}"] = f16(mp["o"][0])
            sh[f"bq_{p}"] = col(mp["q"][1])
            sh[f"bk_{p}"] = col(mp["k"][1])
            sh[f"bv_{p}"] = col(mp["v"][1])
            sh[f"bo_{p}"] = col(mp["o"][1])
        sh[f"we_{l}"] = np.ascontiguousarray(f16(lp["edge_nn"][0]).reshape(3, 128, 128).transpose(1, 0, 2))
        sh[f"be_{l}"] = col(lp["edge_nn"][1])
    return sh


def kernel(context_x, value, mask, target_value, target_mask, exp_stage, params):
    context_x = np.asarray(context_x, np.float32)
    value = np.asarray(value, np.float32)
    mask = np.asarray(mask, np.float32)
    target_value = np.asarray(target_value, np.float32)
    target_mask = np.asarray(target_mask, np.float32)

    nc = _get_program()
    sh = _shared_inputs(params)

    idxs, tvs, tms = [], [], []
    in_maps = []
    for b in range(B):
        mflat = mask[b].reshape(-1)
        idx = np.flatnonzero(mflat > 0)[:N_OBS].astype(np.int64)
        assert idx.shape[0] == N_OBS, f"row {b}: {idx.shape[0]} observed != {N_OBS}"
        idxs.append(idx)
        tvs.append(target_value[b].reshape(-1)[idx])
        tms.append(target_mask[b].reshape(-1)[idx])
        m = dict(sh)
        m.update(_prep_core_inputs(b, context_x, value, target_value, target_mask, idx))
        in_maps.append(m)

    res = run_bass_kernel_spmd(nc, in_maps, list(range(B)), trace=_TRACE)
    if _TRACE:
        kernel.last_results = res

    out = np.stack([res.results[b]["out"][0, :N_OBS] for b in range(B)])[..., None]
    tv_f = np.stack(tvs)
    tm_f = np.stack(tms)
    return (out.astype(np.float32), tv_f.astype(np.float32), tm_f.astype(np.float32))


# revision 37
# speedup vs baseline: 931.1195x; 931.1195x over previous
"""Trainium2 Bass kernel for nn_Encoder_41412074668419 (ragged_sequence).

Strategy: data-parallel over batch B=8 across 8 NeuronCores (one sample per
core).  Host side does only index/layout prep: extracting the ragged-gather
indices from the mask, gathering the per-observation scalars, and building the
one-hot segment matrices (M_t [L,N] / M_c [E,N]) that the device uses to do
gathers and segment-softmax sums as matmuls.

Math reformulation (validated bit-close against the jax reference):
  * The mask has exactly N=2624 ones per row, so the gathered mask is all-ones
    and the attention masks are pure one-hot segment selectors.
  * Masked softmax == segment softmax:
      s[n,h] = (lin(gathered_emb, q) . k[n])_h / sqrt(NK)     (per-observation)
      P = exp(s)                      (scores are tiny; no max subtraction)
      denom[seg,h] = sum_{n in seg} P[n,h]      == M @ P      (matmul)
      numer[seg,:] = sum_{n in seg} P[n,h]*v[n] == M @ (P*v)  (matmul)
      o = q + numer/denom + bias_v              (v-bias rides through softmax)
  * Row gathers emb[idx] are one-hot matmuls; linear biases ride through them.

All matmul operands are fp16 (PE 1 cyc/row, fp32 PSUM accumulate).
"""

import numpy as np

import concourse.bacc as bacc
import concourse.bass as bass
import concourse.mybir as mybir
import concourse.tile as tile
from concourse.bass_utils import run_bass_kernel_spmd

B, L, E = 8, 256, 41
NK, NH, NL = 128, 4, 3
HD = NK // NH
N_OBS = 2624
NP = 2688                 # padded observation count: 21 * 128
NCH = NP // 128           # 21 partition chunks of the n dimension
INV_SQRT_NK = 1.0 / float(np.sqrt(NK))
F16 = mybir.dt.float16
F32 = mybir.dt.float32
ID = mybir.ActivationFunctionType.Identity
COLS = [(o, min(512, NP - o)) for o in range(0, NP, 512)]   # 512-wide col chunks

_TRACE = False            # test.py flips this to profile
_PROGRAM = {}             # iters -> compiled program


def _build_program(iters=1):
    nc = bacc.Bacc("TRN2", target_bir_lowering=False, debug=False)

    def din(name, shape, dt=F16):
        return nc.declare_dram_parameter(name, list(shape), dt, isOutput=False)

    # ---- per-core ragged data ----
    d_vfeat = din("vfeat", [2, NP])                    # [value_f; tm_f]
    d_cx = din("cx", [1, L])
    d_mt = din("mt", [128, 2, NP])                     # M_t   [t_in_chunk, t_chunk, n]
    d_mtt = din("mtt", [128, NCH, L])                  # M_t^T [n_in_chunk, n_chunk, t]
    d_mc = din("mc", [E, NP])                          # M_c   [e, n]
    d_mct = din("mct", [128, NCH, E])                  # M_c^T [n_in_chunk, n_chunk, e]
    # ---- replicated consts ----
    d_ce0b = din("ce0b", [128, E])                     # initial chan emb, [f, e]
    d_ce0a = din("ce0a", [E, 128])                     # initial chan emb, [e, f]
    d_ident = din("ident", [128, 128])
    d_onesh = din("onesh", [128, NH])                  # ones_heads[f, h] = (f//HD == h)
    d_oneshT = din("oneshT", [NH, 128])
    d_einit_w = din("einit_w", [2, 128])
    d_einit_b = din("einit_b", [128, 1], F32)
    d_time_w = din("time_w", [1, 128])
    d_time_b = din("time_b", [128, 1], F32)
    d_outp_w = din("outp_w", [128, 3])                 # W_out [384,1] chunked [c*128+p] -> [p, c]
    d_outp_b = din("outp_b", [1, 1], F32)
    mabs = {}
    for l in range(NL):
        for m in ("ct", "tc"):
            p = f"{m}{l}"
            mabs[p] = dict(
                wq=din(f"wq_{p}", [128, 128]),
                wk=din(f"wk_{p}", [128, 2, 128]),      # [f_in_half, half, f']
                wv=din(f"wv_{p}", [128, 2, 128]),
                wo=din(f"wo_{p}", [128, 128]),
                bq=din(f"bq_{p}", [128, 1], F32),
                bk=din(f"bk_{p}", [128, 1], F32),
                bv=din(f"bv_{p}", [128, 1], F32),
                bo=din(f"bo_{p}", [128, 1], F32),
            )
    d_we = [din(f"we_{l}", [128, 3, 128]) for l in range(NL)]
    d_be = [din(f"be_{l}", [128, 1], F32) for l in range(NL)]

    d_out = nc.declare_dram_parameter("out", [1, NP], F32, isOutput=True)

    # PSUM is 8 banks; every psum tile occupies >=1 full bank:
    #   "mm" 4 + "sm" 2 + numer 1 + denom 1 = 8
    with tile.TileContext(nc) as tc, \
         tc.tile_pool(name="consts", bufs=1) as consts, \
         tc.tile_pool(name="acts", bufs=2) as acts, \
         tc.tile_pool(name="small", bufs=3) as small, \
         tc.tile_pool(name="psum", bufs=5, space="PSUM") as psum, \
         tc.tile_pool(name="psum_sm", bufs=1, space="PSUM") as psum_sm, \
         tc.tile_pool(name="psum_n", bufs=1, space="PSUM") as psum_n, \
         tc.tile_pool(name="psum_d", bufs=1, space="PSUM") as psum_d:

        # Small init-critical tensors first on the SP HWDGE ring; the big
        # one-hot masks go through GPSIMD's SWDGE queues (Pool is idle), so
        # they don't serialize behind or in front of anything.
        def cload(dram, shape, dt=F16, tag=None, big=False):
            t = consts.tile(shape, dt, tag=tag or dram.name)
            eng = nc.gpsimd if big else nc.sync
            eng.dma_start(out=t[:], in_=dram[:])
            return t

        # ---------- load constants ----------
        cx = cload(d_cx, [1, L])
        time_w = cload(d_time_w, [1, 128])
        time_b = cload(d_time_b, [128, 1], F32)
        einit_w = cload(d_einit_w, [2, 128])
        einit_b = cload(d_einit_b, [128, 1], F32)
        vfeat = cload(d_vfeat, [2, NP])
        ce0b = cload(d_ce0b, [128, E])
        ce0a = cload(d_ce0a, [E, 128])
        ident = cload(d_ident, [128, 128])
        onesh = cload(d_onesh, [128, NH])
        oneshT = cload(d_oneshT, [NH, 128])
        mt = cload(d_mt, [128, 2, NP], big=True)
        mc = cload(d_mc, [E, NP], big=True)
        mtt = cload(d_mtt, [128, NCH, L], big=True)
        mct = cload(d_mct, [128, NCH, E], big=True)
        W = {}
        we, be = [None] * NL, [None] * NL
        for l in range(NL):
            for m in ("ct", "tc"):
                p = f"{m}{l}"
                W[p] = {k: cload(v, list(v.shape), v.dtype)
                        for k, v in mabs[p].items()}
            we[l] = cload(d_we[l], [128, 3, 128])
            be[l] = cload(d_be[l], [128, 1], F32)
        outp_w = cload(d_outp_w, [128, 3])
        outp_b = cload(d_outp_b, [1, 1], F32)

        AF = mybir.ActivationFunctionType
        OP = mybir.AluOpType

        # Benchmark mode: repeat the whole compute body (consts stay loaded)
        # so device time can be recovered by wall-clock differencing.
        import contextlib
        _loop = contextlib.ExitStack()
        if iters > 1:
            _loop.enter_context(tc.For_i(0, iters, 1, hint_engines=(
                mybir.EngineType.PE, mybir.EngineType.DVE,
                mybir.EngineType.Activation, mybir.EngineType.SP,
                mybir.EngineType.Pool)))

        # ---------- init ----------
        # time emb: sin(cx @ W + b)   [f, t] and transposed [t, f]
        te_b = acts.tile([128, L], F16, tag="te_b")
        ps = psum_sm.tile([128, L], F32, tag="sm")
        nc.tensor.matmul(ps[:], lhsT=time_w[:], rhs=cx[:], start=True, stop=True)
        nc.scalar.activation(out=te_b[:], in_=ps[:], func=AF.Sin, bias=time_b[:])

        def transpose_b_to_a(src_b, S, tag):
            """[128, S] -> [S(part chunks), 128] as tile [128, ceil(S/128), 128]."""
            nchunk = (S + 127) // 128
            dst = acts.tile([128, nchunk, 128], F16, tag=tag)
            for c in range(nchunk):
                w = min(128, S - c * 128)
                tp = psum_sm.tile([128, 128], F16, tag="sm")
                nc.tensor.transpose(
                    tp[:w, :], in_=src_b[:, c * 128:c * 128 + w], identity=ident[:]
                )
                nc.vector.tensor_copy(out=dst[:w, c, :], in_=tp[:w, :])
            return dst

        te_a = transpose_b_to_a(te_b, L, "te_a")
        ce_b = ce0b
        ce_a_ap = ce0a[:]

        # value feature init: relu(vfeat @ W + b), [f, n]
        vf = acts.tile([128, NP], F16, tag="vf")
        for o, w in COLS:
            ps = psum.tile([128, 512], F32, tag="mm")
            nc.tensor.matmul(ps[:, :w], lhsT=einit_w[:], rhs=vfeat[:, o:o + w],
                             start=True, stop=True)
            nc.scalar.activation(out=vf[:, o:o + w], in_=ps[:, :w], func=AF.Relu,
                                 bias=einit_b[:])

        def gather_time(src_a, tag):
            """k_t[f, n] = sum_t src_a[t, f] * M_t[t, n]"""
            out = acts.tile([128, NP], F16, tag=tag)
            for o, w in COLS:
                ps = psum.tile([128, 512], F32, tag="mm")
                for c in range(2):
                    nc.tensor.matmul(ps[:, :w], lhsT=src_a[:, c, :],
                                     rhs=mt[:, c, o:o + w],
                                     start=(c == 0), stop=(c == 1))
                nc.scalar.copy(out=out[:, o:o + w], in_=ps[:, :w])
            return out

        def gather_chan(src_a_ap, tag):
            """k_c[f, n] = sum_e src_a[e, f] * M_c[e, n];  src_a_ap: [E, 128]."""
            out = acts.tile([128, NP], F16, tag=tag)
            for o, w in COLS:
                ps = psum.tile([128, 512], F32, tag="mm")
                nc.tensor.matmul(ps[:, :w], lhsT=src_a_ap,
                                 rhs=mc[:, o:o + w], start=True, stop=True)
                nc.vector.tensor_copy(out=out[:, o:o + w], in_=ps[:, :w])
            return out

        def mab(wp, q_emb_b, S, qg_src, k_first, m_T):
            """One masked-attention block (segment form).

            wp: weight dict.  q_emb_b: [128, S] query-side embedding (layout b).
            qg_src: [128, NP] query emb gathered per observation.
            k_first: [128, NP] first half of the K-side concat (second is vf).
            m_T: [128, NCH, S] one-hot M^T for numer/denom contraction.
            Returns new embedding in both layouts.
            """
            # q = lin(q_emb) (+bq)
            q_sb = small.tile([128, S], F16, tag="q_sb")
            ps = psum_sm.tile([128, S], F32, tag="sm")
            nc.tensor.matmul(ps[:], lhsT=wp["wq"][:], rhs=q_emb_b[:, :S],
                             start=True, stop=True)
            nc.scalar.activation(out=q_sb[:], in_=ps[:], func=ID, bias=wp["bq"][:])

            # qg = lin(gathered q emb) (+bq)  [f, n]
            qg = acts.tile([128, NP], F16, tag="qg")
            for o, w in COLS:
                ps = psum.tile([128, 512], F32, tag="mm")
                nc.tensor.matmul(ps[:, :w], lhsT=wp["wq"][:], rhs=qg_src[:, o:o + w],
                                 start=True, stop=True)
                nc.scalar.activation(out=qg[:, o:o + w], in_=ps[:, :w], func=ID,
                                     bias=wp["bq"][:])

            # k = lin([k_first; vf]) ; prod = (k + bk) * qg  [f, n]
            prod = acts.tile([128, NP], F16, tag="prod")
            for o, w in COLS:
                ps = psum.tile([128, 512], F32, tag="mm")
                nc.tensor.matmul(ps[:, :w], lhsT=wp["wk"][:, 0, :],
                                 rhs=k_first[:, o:o + w], start=True, stop=False)
                nc.tensor.matmul(ps[:, :w], lhsT=wp["wk"][:, 1, :],
                                 rhs=vf[:, o:o + w], start=False, stop=True)
                nc.vector.scalar_tensor_tensor(out=prod[:, o:o + w], in0=ps[:, :w],
                                               scalar=wp["bk"][:], in1=qg[:, o:o + w],
                                               op0=OP.add, op1=OP.mult)

            # P = exp(s/sqrt(NK)) directly in layout a [n(part chunks), h]:
            # s_a chunk = prod_chunk.T @ ones_heads, 4 chunks per psum bank,
            # one fused exp per group.
            pa = small.tile([128, NCH, NH], F16, tag="pa")
            for g0 in range(0, NCH, 4):
                gn = min(4, NCH - g0)
                ps_sa = psum_sm.tile([128, 4, NH], F32, tag="sm")
                for j in range(gn):
                    c = g0 + j
                    nc.tensor.matmul(ps_sa[:, j, :], lhsT=prod[:, c * 128:(c + 1) * 128],
                                     rhs=onesh[:], start=True, stop=True)
                nc.scalar.activation(out=pa[:, g0:g0 + gn, :], in_=ps_sa[:, :gn, :],
                                     func=AF.Exp, scale=INV_SQRT_NK)

            # v in layout a (bias bv applied post-softmax), fused with PV = v * P;
            # 4 n-chunks share one psum bank so the DVE multiply runs 512 wide
            pv = acts.tile([128, NCH, 128], F16, tag="pv")
            for g0 in range(0, NCH, 4):
                gn = min(4, NCH - g0)
                ps = psum.tile([128, 4, 128], F32, tag="mm")
                for j in range(gn):
                    c = g0 + j
                    nc.tensor.matmul(ps[:, j, :], lhsT=k_first[:, c * 128:(c + 1) * 128],
                                     rhs=wp["wv"][:, 0, :], start=True, stop=False)
                    nc.tensor.matmul(ps[:, j, :], lhsT=vf[:, c * 128:(c + 1) * 128],
                                     rhs=wp["wv"][:, 1, :], start=False, stop=True)
                nc.vector.tensor_tensor(
                    out=pv[:, g0:g0 + gn, :].rearrange("p c (h d) -> p c h d", h=NH),
                    in0=ps[:, :gn, :].rearrange("p c (h d) -> p c h d", h=NH),
                    in1=pa[:, g0:g0 + gn, :, None].to_broadcast((128, gn, NH, HD)),
                    op=OP.mult)

            # numer [f', seg] and denom [h, seg] via one-hot contraction over n
            ps_n = psum_n.tile([128, 256], F32, tag="numer")
            ps_d = psum_d.tile([NH, 256], F32, tag="denom")
            for c in range(NCH):
                nc.tensor.matmul(ps_n[:, :S], lhsT=pv[:, c, :], rhs=m_T[:, c, :S],
                                 start=(c == 0), stop=(c == NCH - 1))
            for c in range(NCH):
                nc.tensor.matmul(ps_d[:, :S], lhsT=pa[:, c, :], rhs=m_T[:, c, :S],
                                 start=(c == 0), stop=(c == NCH - 1))

            # o = q + numer/denom + bv ; out = o + relu(lin(o) + bo)
            rd = small.tile([NH, S], F16, tag="rd")
            with nc.allow_low_precision("softmax denoms are O(segment size); fp16 ok"):
                nc.vector.reciprocal(out=rd[:], in_=ps_d[:, :S])
            ps_rb = psum_sm.tile([128, S], F32, tag="sm")
            nc.tensor.matmul(ps_rb[:], lhsT=oneshT[:], rhs=rd[:], start=True, stop=True)
            rb = small.tile([128, S], F16, tag="rb")
            nc.vector.tensor_copy(out=rb[:], in_=ps_rb[:])
            t0 = small.tile([128, S], F32, tag="t0")
            nc.vector.tensor_mul(t0[:], ps_n[:, :S], rb[:])
            o_pre = small.tile([128, S], F16, tag="o_pre")
            nc.vector.scalar_tensor_tensor(out=o_pre[:], in0=t0[:], scalar=wp["bv"][:],
                                           in1=q_sb[:], op0=OP.add, op1=OP.add)
            ps_o = psum_sm.tile([128, S], F32, tag="sm")
            nc.tensor.matmul(ps_o[:], lhsT=wp["wo"][:], rhs=o_pre[:], start=True, stop=True)
            t1 = small.tile([128, S], F16, tag="t1")
            nc.scalar.activation(out=t1[:], in_=ps_o[:], func=AF.Relu, bias=wp["bo"][:])
            new_b = acts.tile([128, S], F16, tag="new_b")
            nc.vector.tensor_add(new_b[:], o_pre[:], t1[:])
            new_a = transpose_b_to_a(new_b, S, "new_a")
            return new_b, new_a

        # ---------- layers ----------
        for l in range(NL):
            k_t = gather_time(te_a, "k_t")
            k_c = gather_chan(ce_a_ap, "k_c")

            t_b, t_a = mab(W[f"tc{l}"], te_b, L, k_t, k_c, mtt)
            c_b, c_a = mab(W[f"ct{l}"], ce_b, E, k_c, k_t, mct)

            # vf = relu(vf + lin([vf; k_t; k_c]) + be)
            vf_new = acts.tile([128, NP], F16, tag="vf")
            srcs = (vf, k_t, k_c)
            for o, w in COLS:
                ps = psum.tile([128, 512], F32, tag="mm")
                for c in range(3):
                    nc.tensor.matmul(ps[:, :w], lhsT=we[l][:, c, :],
                                     rhs=srcs[c][:, o:o + w],
                                     start=(c == 0), stop=(c == 2))
                t2 = small.tile([128, 512], F32, tag="t_edge")
                nc.vector.tensor_add(t2[:, :w], ps[:, :w], vf[:, o:o + w])
                nc.scalar.activation(out=vf_new[:, o:o + w], in_=t2[:, :w],
                                     func=AF.Relu, bias=be[l][:])
            vf = vf_new
            ce_b = c_b
            ce_a_ap = c_a[:E, 0, :]
            te_b, te_a = t_b, t_a

        # ---------- output ----------
        k_t = gather_time(te_a, "k_t")
        k_c = gather_chan(ce_a_ap, "k_c")
        out_sb = acts.tile([1, NP], F32, tag="out_sb")
        srcs = (vf, k_t, k_c)
        for o, w in COLS:
            ps = psum_sm.tile([1, 512], F32, tag="sm")
            for c in range(3):
                nc.tensor.matmul(ps[:, :w], lhsT=outp_w[:, c:c + 1],
                                 rhs=srcs[c][:, o:o + w],
                                 start=(c == 0), stop=(c == 2))
            nc.scalar.activation(out=out_sb[:, o:o + w], in_=ps[:, :w], func=ID,
                                 bias=outp_b[:])
        nc.sync.dma_start(out=d_out[:], in_=out_sb[:])
        _loop.close()

    nc.compile()
    return nc


def _get_program(iters=1):
    if iters not in _PROGRAM:
        _PROGRAM[iters] = _build_program(iters)
    return _PROGRAM[iters]


def _prep_core_inputs(b, context_x, value, target_value, target_mask, idx):
    tif = idx // E
    cif = idx % E
    n = idx.shape[0]
    vfeat = np.zeros((2, NP), np.float16)
    vfeat[0, :n] = value[b].reshape(-1)[idx]
    vfeat[1, :n] = target_mask[b].reshape(-1)[idx]
    mtf = np.zeros((L, NP), np.float16)
    mtf[tif, np.arange(n)] = 1.0
    mcf = np.zeros((E, NP), np.float16)
    mcf[cif, np.arange(n)] = 1.0
    return {
        "vfeat": vfeat,
        "cx": context_x[b].reshape(1, L).astype(np.float16),
        "mt": np.ascontiguousarray(mtf.reshape(2, 128, NP).transpose(1, 0, 2)),
        "mtt": np.ascontiguousarray(mtf.T.reshape(NCH, 128, L).transpose(1, 0, 2)),
        "mc": mcf,
        "mct": np.ascontiguousarray(mcf.T.reshape(NCH, 128, E).transpose(1, 0, 2)),
    }


def _shared_inputs(params):
    a = lambda x: np.asarray(x)
    f16 = lambda x: np.asarray(x, np.float16)
    col = lambda x: np.asarray(x, np.float32).reshape(-1, 1)
    sh = {}
    wc, bc = params["chan_init"]
    ce0 = np.maximum(a(wc) + a(bc)[None, :], 0.0)           # [E, NK]
    sh["ce0b"] = f16(ce0.T)
    sh["ce0a"] = f16(ce0)
    sh["ident"] = np.eye(128, dtype=np.float16)
    oh = np.zeros((128, NH), np.float16)
    for h in range(NH):
        oh[h * HD:(h + 1) * HD, h] = 1.0
    sh["onesh"] = oh
    sh["oneshT"] = np.ascontiguousarray(oh.T)
    sh["einit_w"] = f16(params["edge_init"][0])
    sh["einit_b"] = col(params["edge_init"][1])
    sh["time_w"] = f16(params["time_init"][0])
    sh["time_b"] = col(params["time_init"][1])
    sh["outp_w"] = np.ascontiguousarray(f16(params["output"][0]).reshape(3, 128).T)
    sh["outp_b"] = col(params["output"][1])
    for l, lp in enumerate(params["layers"]):
        for m in ("ct", "tc"):
            mp, p = lp[m], f"{m}{l}"
            sh[f"wq_{p}"] = f16(mp["q"][0])
            sh[f"wk_{p}"] = np.ascontiguousarray(f16(mp["k"][0]).reshape(2, 128, 128).transpose(1, 0, 2))
            sh[f"wv_{p}"] = np.ascontiguousarray(f16(mp["v"][0]).reshape(2, 128, 128).transpose(1, 0, 2))
            sh[f"wo_{p}"] = f16(mp["o"][0])
            sh[f"bq_{p}"] = col(mp["q"][1])
            sh[f"bk_{p}"] = col(mp["k"][1])
            sh[f"bv_{p}"] = col(mp["v"][1])
            sh[f"bo_{p}"] = col(mp["o"][1])
        sh[f"we_{l}"] = np.ascontiguousarray(f16(lp["edge_nn"][0]).reshape(3, 128, 128).transpose(1, 0, 2))
        sh[f"be_{l}"] = col(lp["edge_nn"][1])
    return sh


def kernel(context_x, value, mask, target_value, target_mask, exp_stage, params):
    context_x = np.asarray(context_x, np.float32)
    value = np.asarray(value, np.float32)
    mask = np.asarray(mask, np.float32)
    target_value = np.asarray(target_value, np.float32)
    target_mask = np.asarray(target_mask, np.float32)

    nc = _get_program()
    sh = _shared_inputs(params)

    idxs, tvs, tms = [], [], []
    in_maps = []
    for b in range(B):
        mflat = mask[b].reshape(-1)
        idx = np.flatnonzero(mflat > 0)[:N_OBS].astype(np.int64)
        assert idx.shape[0] == N_OBS, f"row {b}: {idx.shape[0]} observed != {N_OBS}"
        idxs.append(idx)
        tvs.append(target_value[b].reshape(-1)[idx])
        tms.append(target_mask[b].reshape(-1)[idx])
        m = dict(sh)
        m.update(_prep_core_inputs(b, context_x, value, target_value, target_mask, idx))
        in_maps.append(m)

    res = run_bass_kernel_spmd(nc, in_maps, list(range(B)), trace=_TRACE)
    if _TRACE:
        kernel.last_results = res

    out = np.stack([res.results[b]["out"][0, :N_OBS] for b in range(B)])[..., None]
    tv_f = np.stack(tvs)
    tm_f = np.stack(tms)
    return (out.astype(np.float32), tv_f.astype(np.float32), tm_f.astype(np.float32))


# revision 41
# speedup vs baseline: 1123.6128x; 1.2067x over previous
"""Trainium2 Bass kernel for nn_Encoder_41412074668419 (ragged_sequence).

Strategy: data-parallel over batch B=8 across 8 NeuronCores (one sample per
core).  Host side does only index/layout prep: extracting the ragged-gather
indices from the mask, gathering the per-observation scalars, and building the
one-hot segment matrices (M_t [L,N] / M_c [E,N]) that the device uses to do
gathers and segment-softmax sums as matmuls.

Math reformulation (validated bit-close against the jax reference):
  * The mask has exactly N=2624 ones per row, so the gathered mask is all-ones
    and the attention masks are pure one-hot segment selectors.
  * Masked softmax == segment softmax:
      s[n,h] = (lin(gathered_emb, q) . k[n])_h / sqrt(NK)     (per-observation)
      P = exp(s)                      (scores are tiny; no max subtraction)
      denom[seg,h] = sum_{n in seg} P[n,h]      == M @ P      (matmul)
      numer[seg,:] = sum_{n in seg} P[n,h]*v[n] == M @ (P*v)  (matmul)
      o = q + numer/denom + bias_v              (v-bias rides through softmax)
  * Row gathers emb[idx] are one-hot matmuls; linear biases ride through them.

All matmul operands are fp16 (PE 1 cyc/row, fp32 PSUM accumulate).
"""

import numpy as np

import concourse.bacc as bacc
import concourse.bass as bass
import concourse.mybir as mybir
import concourse.tile as tile
from concourse.bass_utils import run_bass_kernel_spmd

B, L, E = 8, 256, 41
NK, NH, NL = 128, 4, 3
HD = NK // NH
N_OBS = 2624
NP = 2688                 # padded observation count: 21 * 128
NCH = NP // 128           # 21 partition chunks of the n dimension
INV_SQRT_NK = 1.0 / float(np.sqrt(NK))
F16 = mybir.dt.float16
F32 = mybir.dt.float32
ID = mybir.ActivationFunctionType.Identity
COLS = [(o, min(512, NP - o)) for o in range(0, NP, 512)]   # 512-wide col chunks

_TRACE = False            # test.py flips this to profile
_PROGRAM = {}             # iters -> compiled program


def _build_program(iters=1):
    nc = bacc.Bacc("TRN2", target_bir_lowering=False, debug=False)

    def din(name, shape, dt=F16):
        return nc.declare_dram_parameter(name, list(shape), dt, isOutput=False)

    # ---- per-core ragged data ----
    d_vfeat = din("vfeat", [2, NP])                    # [value_f; tm_f]
    d_cx = din("cx", [1, L])
    d_mt = din("mt", [128, 2, NP])                     # M_t   [t_in_chunk, t_chunk, n]
    d_mtt = din("mtt", [128, NCH, L])                  # M_t^T [n_in_chunk, n_chunk, t]
    d_mc = din("mc", [E, NP])                          # M_c   [e, n]
    d_mct = din("mct", [128, NCH, E])                  # M_c^T [n_in_chunk, n_chunk, e]
    # ---- replicated consts ----
    d_ce0b = din("ce0b", [128, E])                     # initial chan emb, [f, e]
    d_ce0a = din("ce0a", [E, 128])                     # initial chan emb, [e, f]
    d_ident = din("ident", [128, 128])
    d_onesh = din("onesh", [128, NH])                  # ones_heads[f, h] = (f//HD == h)
    d_oneshT = din("oneshT", [NH, 128])
    d_einit_w = din("einit_w", [2, 128])
    d_einit_b = din("einit_b", [128, 1], F32)
    d_time_w = din("time_w", [1, 128])
    d_time_b = din("time_b", [128, 1], F32)
    d_outp_w = din("outp_w", [128, 3])                 # W_out [384,1] chunked [c*128+p] -> [p, c]
    d_outp_b = din("outp_b", [1, 1], F32)
    mabs = {}
    for l in range(NL):
        for m in ("ct", "tc"):
            p = f"{m}{l}"
            mabs[p] = dict(
                wq=din(f"wq_{p}", [128, 128]),
                wk=din(f"wk_{p}", [128, 2, 128]),      # [f_in_half, half, f']
                wv=din(f"wv_{p}", [128, 2, 128]),
                wo=din(f"wo_{p}", [128, 128]),
                bq=din(f"bq_{p}", [128, 1], F32),
                bk=din(f"bk_{p}", [128, 1], F32),
                bv=din(f"bv_{p}", [128, 1], F32),
                bo=din(f"bo_{p}", [128, 1], F32),
            )
    d_we = [din(f"we_{l}", [128, 3, 128]) for l in range(NL)]
    d_be = [din(f"be_{l}", [128, 1], F32) for l in range(NL)]

    d_out = nc.declare_dram_parameter("out", [1, NP], F32, isOutput=True)

    # PSUM is 8 banks; every psum tile occupies >=1 full bank:
    #   "mm" 4 + "sm" 2 + numer 1 + denom 1 = 8
    with tile.TileContext(nc) as tc, \
         tc.tile_pool(name="consts", bufs=1) as consts, \
         tc.tile_pool(name="acts", bufs=2) as acts, \
         tc.tile_pool(name="small", bufs=3) as small, \
         tc.tile_pool(name="psum", bufs=5, space="PSUM") as psum, \
         tc.tile_pool(name="psum_sm", bufs=1, space="PSUM") as psum_sm, \
         tc.tile_pool(name="psum_n", bufs=1, space="PSUM") as psum_n, \
         tc.tile_pool(name="psum_d", bufs=1, space="PSUM") as psum_d:

        # Small init-critical tensors first on the SP HWDGE ring; the big
        # one-hot masks go through GPSIMD's SWDGE queues (Pool is idle), so
        # they don't serialize behind or in front of anything.
        def cload(dram, shape, dt=F16, tag=None, big=False):
            t = consts.tile(shape, dt, tag=tag or dram.name)
            eng = nc.gpsimd if big else nc.sync
            eng.dma_start(out=t[:], in_=dram[:])
            return t

        # ---------- load constants ----------
        cx = cload(d_cx, [1, L])
        time_w = cload(d_time_w, [1, 128])
        time_b = cload(d_time_b, [128, 1], F32)
        einit_w = cload(d_einit_w, [2, 128])
        einit_b = cload(d_einit_b, [128, 1], F32)
        vfeat = cload(d_vfeat, [2, NP])
        ce0b = cload(d_ce0b, [128, E])
        ce0a = cload(d_ce0a, [E, 128])
        ident = cload(d_ident, [128, 128])
        onesh = cload(d_onesh, [128, NH])
        oneshT = cload(d_oneshT, [NH, 128])
        mt = cload(d_mt, [128, 2, NP], big=True)
        mc = cload(d_mc, [E, NP], big=True)
        mtt = cload(d_mtt, [128, NCH, L], big=True)
        mct = cload(d_mct, [128, NCH, E], big=True)
        W = {}
        we, be = [None] * NL, [None] * NL
        for l in range(NL):
            for m in ("ct", "tc"):
                p = f"{m}{l}"
                W[p] = {k: cload(v, list(v.shape), v.dtype)
                        for k, v in mabs[p].items()}
            we[l] = cload(d_we[l], [128, 3, 128])
            be[l] = cload(d_be[l], [128, 1], F32)
        outp_w = cload(d_outp_w, [128, 3])
        outp_b = cload(d_outp_b, [1, 1], F32)

        AF = mybir.ActivationFunctionType
        OP = mybir.AluOpType

        # Benchmark mode: repeat the whole compute body (consts stay loaded)
        # so device time can be recovered by wall-clock differencing.
        import contextlib
        _loop = contextlib.ExitStack()
        if iters > 1:
            _loop.enter_context(tc.For_i(0, iters, 1, hint_engines=(
                mybir.EngineType.PE, mybir.EngineType.DVE,
                mybir.EngineType.Activation, mybir.EngineType.SP,
                mybir.EngineType.Pool)))

        # ---------- init ----------
        # time emb: sin(cx @ W + b)   [f, t] and transposed [t, f]
        te_b = acts.tile([128, L], F16, tag="te_b")
        ps = psum_sm.tile([128, L], F32, tag="sm")
        nc.tensor.matmul(ps[:], lhsT=time_w[:], rhs=cx[:], start=True, stop=True)
        nc.scalar.activation(out=te_b[:], in_=ps[:], func=AF.Sin, bias=time_b[:])

        def transpose_b_to_a(src_b, S, tag):
            """[128, S] -> [S(part chunks), 128] as tile [128, ceil(S/128), 128]."""
            nchunk = (S + 127) // 128
            dst = acts.tile([128, nchunk, 128], F16, tag=tag)
            for c in range(nchunk):
                w = min(128, S - c * 128)
                tp = psum_sm.tile([128, 128], F16, tag="sm")
                nc.tensor.transpose(
                    tp[:w, :], in_=src_b[:, c * 128:c * 128 + w], identity=ident[:]
                )
                nc.vector.tensor_copy(out=dst[:w, c, :], in_=tp[:w, :])
            return dst

        te_a = transpose_b_to_a(te_b, L, "te_a")
        ce_b = ce0b
        ce_a_ap = ce0a[:]

        # value feature init: relu(vfeat @ W + b), [f, n]
        vf = acts.tile([128, NP], F16, tag="vf")
        for o, w in COLS:
            ps = psum.tile([128, 512], F32, tag="mm")
            nc.tensor.matmul(ps[:, :w], lhsT=einit_w[:], rhs=vfeat[:, o:o + w],
                             start=True, stop=True)
            nc.scalar.activation(out=vf[:, o:o + w], in_=ps[:, :w], func=AF.Relu,
                                 bias=einit_b[:])

        def gather_time(src_a, tag):
            """k_t[f, n] = sum_t src_a[t, f] * M_t[t, n]"""
            out = acts.tile([128, NP], F16, tag=tag)
            for o, w in COLS:
                ps = psum.tile([128, 512], F32, tag="mm")
                for c in range(2):
                    nc.tensor.matmul(ps[:, :w], lhsT=src_a[:, c, :],
                                     rhs=mt[:, c, o:o + w],
                                     start=(c == 0), stop=(c == 1))
                nc.scalar.copy(out=out[:, o:o + w], in_=ps[:, :w])
            return out

        def gather_chan(src_a_ap, tag):
            """k_c[f, n] = sum_e src_a[e, f] * M_c[e, n];  src_a_ap: [E, 128]."""
            out = acts.tile([128, NP], F16, tag=tag)
            for o, w in COLS:
                ps = psum.tile([128, 512], F32, tag="mm")
                nc.tensor.matmul(ps[:, :w], lhsT=src_a_ap,
                                 rhs=mc[:, o:o + w], start=True, stop=True)
                nc.vector.tensor_copy(out=out[:, o:o + w], in_=ps[:, :w])
            return out

        def mab(wp, q_emb_b, S, qg_src, k_first, m_T):
            """One masked-attention block (segment form).

            wp: weight dict.  q_emb_b: [128, S] query-side embedding (layout b).
            qg_src: [128, NP] query emb gathered per observation.
            k_first: [128, NP] first half of the K-side concat (second is vf).
            m_T: [128, NCH, S] one-hot M^T for numer/denom contraction.
            Returns new embedding in both layouts.
            """
            # q = lin(q_emb) (+bq)
            q_sb = small.tile([128, S], F16, tag="q_sb")
            ps = psum_sm.tile([128, S], F32, tag="sm")
            nc.tensor.matmul(ps[:], lhsT=wp["wq"][:], rhs=q_emb_b[:, :S],
                             start=True, stop=True)
            nc.scalar.activation(out=q_sb[:], in_=ps[:], func=ID, bias=wp["bq"][:])

            # qg = lin(gathered q emb) (+bq)  [f, n]
            qg = acts.tile([128, NP], F16, tag="qg")
            for o, w in COLS:
                ps = psum.tile([128, 512], F32, tag="mm")
                nc.tensor.matmul(ps[:, :w], lhsT=wp["wq"][:], rhs=qg_src[:, o:o + w],
                                 start=True, stop=True)
                nc.scalar.activation(out=qg[:, o:o + w], in_=ps[:, :w], func=ID,
                                     bias=wp["bq"][:])

            # k = lin([k_first; vf]) ; prod = (k + bk) * qg  [f, n]
            prod = acts.tile([128, NP], F16, tag="prod")
            for o, w in COLS:
                ps = psum.tile([128, 512], F32, tag="mm")
                nc.tensor.matmul(ps[:, :w], lhsT=wp["wk"][:, 0, :],
                                 rhs=k_first[:, o:o + w], start=True, stop=False)
                nc.tensor.matmul(ps[:, :w], lhsT=wp["wk"][:, 1, :],
                                 rhs=vf[:, o:o + w], start=False, stop=True)
                nc.vector.scalar_tensor_tensor(out=prod[:, o:o + w], in0=ps[:, :w],
                                               scalar=wp["bk"][:], in1=qg[:, o:o + w],
                                               op0=OP.add, op1=OP.mult)

            # P = exp(s/sqrt(NK)) directly in layout a [n(part chunks), h]:
            # s_a chunk = prod_chunk.T @ ones_heads, 4 chunks per psum bank,
            # one fused exp per group.
            pa = small.tile([128, NCH, NH], F16, tag="pa")
            for g0 in range(0, NCH, 4):
                gn = min(4, NCH - g0)
                ps_sa = psum_sm.tile([128, 4, NH], F32, tag="sm")
                for j in range(gn):
                    c = g0 + j
                    nc.tensor.matmul(ps_sa[:, j, :], lhsT=prod[:, c * 128:(c + 1) * 128],
                                     rhs=onesh[:], start=True, stop=True)
                nc.scalar.activation(out=pa[:, g0:g0 + gn, :], in_=ps_sa[:, :gn, :],
                                     func=AF.Exp, scale=INV_SQRT_NK)

            # v in layout a (bias bv applied post-softmax), fused with PV = v * P;
            # 4 n-chunks share one psum bank so the DVE multiply runs 512 wide
            pv = acts.tile([128, NCH, 128], F16, tag="pv")
            for g0 in range(0, NCH, 4):
                gn = min(4, NCH - g0)
                ps = psum.tile([128, 4, 128], F32, tag="mm")
                for j in range(gn):
                    c = g0 + j
                    nc.tensor.matmul(ps[:, j, :], lhsT=k_first[:, c * 128:(c + 1) * 128],
                                     rhs=wp["wv"][:, 0, :], start=True, stop=False)
                    nc.tensor.matmul(ps[:, j, :], lhsT=vf[:, c * 128:(c + 1) * 128],
                                     rhs=wp["wv"][:, 1, :], start=False, stop=True)
                nc.vector.tensor_tensor(
                    out=pv[:, g0:g0 + gn, :].rearrange("p c (h d) -> p c h d", h=NH),
                    in0=ps[:, :gn, :].rearrange("p c (h d) -> p c h d", h=NH),
                    in1=pa[:, g0:g0 + gn, :, None].to_broadcast((128, gn, NH, HD)),
                    op=OP.mult)

            # numer [f', seg] and denom [h, seg] via one-hot contraction over n
            ps_n = psum_n.tile([128, 256], F32, tag="numer")
            ps_d = psum_d.tile([NH, 256], F32, tag="denom")
            for c in range(NCH):
                nc.tensor.matmul(ps_n[:, :S], lhsT=pv[:, c, :], rhs=m_T[:, c, :S],
                                 start=(c == 0), stop=(c == NCH - 1))
            for c in range(NCH):
                nc.tensor.matmul(ps_d[:, :S], lhsT=pa[:, c, :], rhs=m_T[:, c, :S],
                                 start=(c == 0), stop=(c == NCH - 1))

            # o = q + numer/denom + bv ; out = o + relu(lin(o) + bo)
            rd = small.tile([NH, S], F16, tag="rd")
            with nc.allow_low_precision("softmax denoms are O(segment size); fp16 ok"):
                nc.vector.reciprocal(out=rd[:], in_=ps_d[:, :S])
            ps_rb = psum_sm.tile([128, S], F32, tag="sm")
            nc.tensor.matmul(ps_rb[:], lhsT=oneshT[:], rhs=rd[:], start=True, stop=True)
            rb = small.tile([128, S], F16, tag="rb")
            nc.vector.tensor_copy(out=rb[:], in_=ps_rb[:])
            t0 = small.tile([128, S], F32, tag="t0")
            nc.vector.tensor_mul(t0[:], ps_n[:, :S], rb[:])
            o_pre = small.tile([128, S], F16, tag="o_pre")
            nc.vector.scalar_tensor_tensor(out=o_pre[:], in0=t0[:], scalar=wp["bv"][:],
                                           in1=q_sb[:], op0=OP.add, op1=OP.add)
            ps_o = psum_sm.tile([128, S], F32, tag="sm")
            nc.tensor.matmul(ps_o[:], lhsT=wp["wo"][:], rhs=o_pre[:], start=True, stop=True)
            t1 = small.tile([128, S], F16, tag="t1")
            nc.scalar.activation(out=t1[:], in_=ps_o[:], func=AF.Relu, bias=wp["bo"][:])
            new_b = acts.tile([128, S], F16, tag="new_b")
            nc.vector.tensor_add(new_b[:], o_pre[:], t1[:])
            new_a = transpose_b_to_a(new_b, S, "new_a")
            return new_b, new_a

        # ---------- layers ----------
        for l in range(NL):
            k_t = gather_time(te_a, "k_t")
            k_c = gather_chan(ce_a_ap, "k_c")

            t_b, t_a = mab(W[f"tc{l}"], te_b, L, k_t, k_c, mtt)
            c_b, c_a = mab(W[f"ct{l}"], ce_b, E, k_c, k_t, mct)

            # vf = relu(vf + lin([vf; k_t; k_c]) + be)
            vf_new = acts.tile([128, NP], F16, tag="vf")
            srcs = (vf, k_t, k_c)
            for o, w in COLS:
                ps = psum.tile([128, 512], F32, tag="mm")
                for c in range(3):
                    nc.tensor.matmul(ps[:, :w], lhsT=we[l][:, c, :],
                                     rhs=srcs[c][:, o:o + w],
                                     start=(c == 0), stop=(c == 2))
                t2 = small.tile([128, 512], F32, tag="t_edge")
                nc.vector.tensor_add(t2[:, :w], ps[:, :w], vf[:, o:o + w])
                nc.scalar.activation(out=vf_new[:, o:o + w], in_=t2[:, :w],
                                     func=AF.Relu, bias=be[l][:])
            vf = vf_new
            ce_b = c_b
            ce_a_ap = c_a[:E, 0, :]
            te_b, te_a = t_b, t_a

        # ---------- output ----------
        k_t = gather_time(te_a, "k_t")
        k_c = gather_chan(ce_a_ap, "k_c")
        out_sb = acts.tile([1, NP], F32, tag="out_sb")
        srcs = (vf, k_t, k_c)
        for o, w in COLS:
            ps = psum_sm.tile([1, 512], F32, tag="sm")
            for c in range(3):
                nc.tensor.matmul(ps[:, :w], lhsT=outp_w[:, c:c + 1],
                                 rhs=srcs[c][:, o:o + w],
                                 start=(c == 0), stop=(c == 2))
            nc.scalar.activation(out=out_sb[:, o:o + w], in_=ps[:, :w], func=ID,
                                 bias=outp_b[:])
        nc.sync.dma_start(out=d_out[:], in_=out_sb[:])
        _loop.close()

    nc.compile()
    return nc


def _get_program(iters=1):
    if iters not in _PROGRAM:
        _PROGRAM[iters] = _build_program(iters)
    return _PROGRAM[iters]


def _prep_core_inputs(b, context_x, value, target_value, target_mask, idx):
    tif = idx // E
    cif = idx % E
    n = idx.shape[0]
    vfeat = np.zeros((2, NP), np.float16)
    vfeat[0, :n] = value[b].reshape(-1)[idx]
    vfeat[1, :n] = target_mask[b].reshape(-1)[idx]
    mtf = np.zeros((L, NP), np.float16)
    mtf[tif, np.arange(n)] = 1.0
    mcf = np.zeros((E, NP), np.float16)
    mcf[cif, np.arange(n)] = 1.0
    return {
        "vfeat": vfeat,
        "cx": context_x[b].reshape(1, L).astype(np.float16),
        "mt": np.ascontiguousarray(mtf.reshape(2, 128, NP).transpose(1, 0, 2)),
        "mtt": np.ascontiguousarray(mtf.T.reshape(NCH, 128, L).transpose(1, 0, 2)),
        "mc": mcf,
        "mct": np.ascontiguousarray(mcf.T.reshape(NCH, 128, E).transpose(1, 0, 2)),
    }


def _shared_inputs(params):
    a = lambda x: np.asarray(x)
    f16 = lambda x: np.asarray(x, np.float16)
    col = lambda x: np.asarray(x, np.float32).reshape(-1, 1)
    sh = {}
    wc, bc = params["chan_init"]
    ce0 = np.maximum(a(wc) + a(bc)[None, :], 0.0)           # [E, NK]
    sh["ce0b"] = f16(ce0.T)
    sh["ce0a"] = f16(ce0)
    sh["ident"] = np.eye(128, dtype=np.float16)
    oh = np.zeros((128, NH), np.float16)
    for h in range(NH):
        oh[h * HD:(h + 1) * HD, h] = 1.0
    sh["onesh"] = oh
    sh["oneshT"] = np.ascontiguousarray(oh.T)
    sh["einit_w"] = f16(params["edge_init"][0])
    sh["einit_b"] = col(params["edge_init"][1])
    sh["time_w"] = f16(params["time_init"][0])
    sh["time_b"] = col(params["time_init"][1])
    sh["outp_w"] = np.ascontiguousarray(f16(params["output"][0]).reshape(3, 128).T)
    sh["outp_b"] = col(params["output"][1])
    for l, lp in enumerate(params["layers"]):
        for m in ("ct", "tc"):
            mp, p = lp[m], f"{m}{l}"
            sh[f"wq_{p}"] = f16(mp["q"][0])
            sh[f"wk_{p}"] = np.ascontiguousarray(f16(mp["k"][0]).reshape(2, 128, 128).transpose(1, 0, 2))
            sh[f"wv_{p}"] = np.ascontiguousarray(f16(mp["v"][0]).reshape(2, 128, 128).transpose(1, 0, 2))
            sh[f"wo_{p}"] = f16(mp["o"][0])
            sh[f"bq_{p}"] = col(mp["q"][1])
            sh[f"bk_{p}"] = col(mp["k"][1])
            sh[f"bv_{p}"] = col(mp["v"][1])
            sh[f"bo_{p}"] = col(mp["o"][1])
        sh[f"we_{l}"] = np.ascontiguousarray(f16(lp["edge_nn"][0]).reshape(3, 128, 128).transpose(1, 0, 2))
        sh[f"be_{l}"] = col(lp["edge_nn"][1])
    return sh


def kernel(context_x, value, mask, target_value, target_mask, exp_stage, params):
    context_x = np.asarray(context_x, np.float32)
    value = np.asarray(value, np.float32)
    mask = np.asarray(mask, np.float32)
    target_value = np.asarray(target_value, np.float32)
    target_mask = np.asarray(target_mask, np.float32)

    nc = _get_program()
    sh = _shared_inputs(params)

    idxs, tvs, tms = [], [], []
    in_maps = []
    for b in range(B):
        mflat = mask[b].reshape(-1)
        idx = np.flatnonzero(mflat > 0)[:N_OBS].astype(np.int64)
        assert idx.shape[0] == N_OBS, f"row {b}: {idx.shape[0]} observed != {N_OBS}"
        idxs.append(idx)
        tvs.append(target_value[b].reshape(-1)[idx])
        tms.append(target_mask[b].reshape(-1)[idx])
        m = dict(sh)
        m.update(_prep_core_inputs(b, context_x, value, target_value, target_mask, idx))
        in_maps.append(m)

    res = run_bass_kernel_spmd(nc, in_maps, list(range(B)), trace=_TRACE)
    if _TRACE:
        kernel.last_results = res

    out = np.stack([res.results[b]["out"][0, :N_OBS] for b in range(B)])[..., None]
    tv_f = np.stack(tvs)
    tm_f = np.stack(tms)
    return (out.astype(np.float32), tv_f.astype(np.float32), tm_f.astype(np.float32))
